# revision 3
# baseline (speedup 1.0000x reference)
"""Fused dual-softmax attention (nn_Attention sparse_attention) on 8x TRN2.

Sharding: data-parallel over batch -- one batch element per NeuronCore.

Per-core pipeline (feature-major activations, key-major score matrices).
All matmul operands are fp16 (1 cyc/row on PE); PSUM accumulation is fp32.

Four-stage software pipeline over heads, one head per iteration, with the
lidar stage running TWO iterations ahead of the attention stage so the
per-head DMA bounce chains (bc broadcast for the lidar-softmax column
scale, 1/s broadcast for the final softmax normalize) never sit on the
critical path.  This keeps ScalarE (the exp engine, the true roofline of
this kernel: 128 exp passes over [128,1024] tiles ~ 147us) nearly 100%
busy and keeps the PE free of the >1us stalls that caused HAM clock
oscillation (K=4/8 throttling) in earlier revisions.

  qT/kT = w_{q,k} @ x.T            (w0*SCALE pre-folded into wq)
  v     = x @ w_v.T  token-major, augmented with a ones column per head
  iteration it (hl=it lidar, ha=it-2 attention, hf=it-3 finish):
    F(hf):  ot = O_psum[0:64] * brs    (single DVE mult reading PSUM,
                                        frees the O bank) -> merge -> om
    L(hl):  lsim = lid_h.T @ lid_h     (symmetric; sqrt(SCALE) folded)
            explid = exp(lsim - 8)     (ACT; bias keeps fp16 range; shift
                                        cancels in the lidar softmax)
            sl[:,jc] = rowsum(explid)  (DVE X-reduce; symmetry => row sums
                                        equal the key-axis softmax sums)
    A(ha):  Lt = explid * bc           (DVE fp16 2x, in place)
            mid = k_h.T q_h + I @ Lt   (dots MM + identity MM into PSUM)
            E  = exp(mid)              (ACT, PSUM -> SBUF fp16)
            O += [v_h | 1].T @ E       (PE; row 64 = softmax denominators)
            then start 1/s chain: copy s row, bounce to [8,128], recip,
            bounce back, broadcast -> brs (consumed by F next iteration)
    L-tail: bc chain: recip(sl)*w1 -> PE transpose -> DRAM bounce ->
            broadcast bc_t (consumed by A two iterations later)
  outT = w_out.T.T @ om (+ b_out) -> DMA out; host transposes back.

Softmax max-subtraction is dropped (|scores| <= ~20, exp safe) and conv_b
is dropped (softmax is shift-invariant along the reduced axis).
"""

import sys

try:
    import concourse.bass as bass
except ImportError:  # pragma: no cover
    sys.path.insert(0, "/opt/trn_rl_repo")
    import concourse.bass as bass

import numpy as np

import concourse.mybir as mybir
from concourse import bacc
from concourse.tile import TileContext
from concourse.bass_utils import run_bass_kernel_spmd

F32 = mybir.dt.float32
F16 = mybir.dt.float16
AX = mybir.AluOpType
EXP = mybir.ActivationFunctionType.Exp

B, N, DIM, H, DH = 8, 1024, 512, 8, 64
INNER = H * DH          # 512
QK = 2 * INNER          # 1024 (q|k feature rows of w_qkv)
SCALE = DH ** -0.5
LBIAS = -8.0            # lidar-exp shift: keeps exp(lsim) inside fp16 range
P = 128
NH = N // 2             # 512: max matmul free dim / fp32 PSUM bank
KC = DIM // P           # 4 contraction chunks
TC = N // P             # 8 token chunks
VW = DH + 1             # per-head v width incl. ones column
LAG = 2                 # lidar stage runs LAG iterations ahead of attention

_cache = {}


def _build(w1, need_bm, need_bo):
    nc = bacc.Bacc("TRN2", target_bir_lowering=False, debug=False, num_devices=B)

    xT = nc.dram_tensor("xT", [DIM, N], F16, kind="ExternalInput")
    lidT = nc.dram_tensor("lidT", [DIM, N], F16, kind="ExternalInput")
    wqkT = nc.dram_tensor("wqkT", [DIM, QK], F16, kind="ExternalInput")
    wvT = nc.dram_tensor("wvT", [DIM, INNER], F16, kind="ExternalInput")
    wmT = nc.dram_tensor("wmT", [DH, DH], F16, kind="ExternalInput")
    woT = nc.dram_tensor("woT", [INNER, DIM], F16, kind="ExternalInput")
    ident = nc.dram_tensor("ident", [P, P], F16, kind="ExternalInput")
    identf = nc.dram_tensor("identf", [P, P], F32, kind="ExternalInput")
    onesv = nc.dram_tensor("onesv", [P, H, 1], F16, kind="ExternalInput")
    bm = nc.dram_tensor("bm", [DH, 1], F32, kind="ExternalInput")
    bo = nc.dram_tensor("bo", [P, KC], F32, kind="ExternalInput")
    y = nc.dram_tensor("y", [DIM, N], F32, kind="ExternalOutput")

    with TileContext(nc) as tc:
        with (
            tc.tile_pool(name="persist", bufs=1) as pp,
            tc.tile_pool(name="ps_sc", bufs=3, space="PSUM") as ps_sc,
            tc.tile_pool(name="ps_o", bufs=1, space="PSUM") as ps_o,
        ):
            # ---------------- persistent SBUF ----------------
            lid_sb = [pp.tile([P, N], F16, name=f"lid{i}", tag=f"lid{i}") for i in range(KC)]
            qT_sb = [pp.tile([P, N], F16, name=f"qT{i}", tag=f"qT{i}") for i in range(KC)]
            kT_sb = [pp.tile([P, N], F16, name=f"kT{i}", tag=f"kT{i}") for i in range(KC)]
            v_sb = [pp.tile([P, H * VW], F16, name=f"v{i}", tag=f"v{i}") for i in range(TC)]
            om_sb = [pp.tile([P, N], F16, name=f"om{i}", tag=f"om{i}") for i in range(KC)]
            id_sb = pp.tile([P, P], F16, name="ident", tag="ident")
            idf_sb = pp.tile([P, P], F32, name="identf", tag="identf")
            wm_sb = pp.tile([DH, DH], F16, name="wm", tag="wm")
            bm_sb = pp.tile([DH, 1], F32, name="bm", tag="bm")
            bo_sb = pp.tile([P, KC], F32, name="bo", tag="bo")
            lb_sb = pp.tile([P, 1], F32, name="lb", tag="lb")
            nc.vector.memset(lb_sb[:], LBIAS)
            wo_sb = [pp.tile([P, DIM], F16, name=f"wo{i}", tag=f"wo{i}") for i in range(KC)]
            ypar_sb = [pp.tile([P, N], F32, name=f"ypar{i}", tag=f"ypar{i}") for i in range(KC)]

            for c in range(KC):
                nc.sync.dma_start(lid_sb[c][:], lidT[c * P:(c + 1) * P, :])
            nc.sync.dma_start(id_sb[:], ident[:, :])
            nc.sync.dma_start(idf_sb[:], identf[:, :])
            nc.sync.dma_start(wm_sb[:], wmT[:, :])
            nc.sync.dma_start(bm_sb[:], bm[:, :])
            nc.sync.dma_start(bo_sb[:], bo[:, :])
            for kc in range(KC):
                nc.sync.dma_start(wo_sb[kc][:], woT[kc * P:(kc + 1) * P, :])

            # phase-1 loads (projection matmuls interleave into the early
            # pipeline iterations below to keep PE density high)
            lp = ctx_lp = tc.tile_pool(name="load", bufs=1)
            lp = ctx_lp.__enter__()
            x_sb = [lp.tile([P, N], F16, name=f"x{i}", tag=f"x{i}") for i in range(KC)]
            wqk_sb = [lp.tile([P, QK], F16, name=f"wqk{i}", tag=f"wqk{i}") for i in range(KC)]
            wv_sb = [lp.tile([P, INNER], F16, name=f"wv{i}", tag=f"wv{i}") for i in range(KC)]
            for c in range(KC):
                nc.sync.dma_start(x_sb[c][:], xT[c * P:(c + 1) * P, :])
                nc.sync.dma_start(wqk_sb[c][:], wqkT[c * P:(c + 1) * P, :])
                nc.sync.dma_start(wv_sb[c][:], wvT[c * P:(c + 1) * P, :])

            def emit_qk_group(fc):
                # qT|kT feature-major: out[fc,:] = sum_kc wqk[kc,fc].T @ xT[kc,:]
                dst = (qT_sb if fc < KC else kT_sb)[fc % KC]
                for ih in range(2):
                    pt = ps_sc.tile([P, NH], F32, name="w", tag="w")
                    for kc in range(KC):
                        nc.tensor.matmul(
                            pt[:],
                            wqk_sb[kc][:, fc * P:(fc + 1) * P],
                            x_sb[kc][:, ih * NH:(ih + 1) * NH],
                            start=(kc == 0), stop=(kc == KC - 1),
                        )
                    nc.vector.tensor_copy(dst[:, ih * NH:(ih + 1) * NH], pt[:])

            def emit_v_group(t):
                # v token-major: v[t,:] = sum_kc xT[kc,t].T @ wvT[kc,:]
                pt = ps_sc.tile([P, INNER], F32, name="w", tag="w")
                for kc in range(KC):
                    nc.tensor.matmul(
                        pt[:],
                        x_sb[kc][:, t * P:(t + 1) * P],
                        wv_sb[kc][:],
                        start=(kc == 0), stop=(kc == KC - 1),
                    )
                v3 = v_sb[t][:].rearrange("p (h w) -> p h w", h=H)
                nc.vector.tensor_copy(
                    v3[:, :, 0:DH], pt[:].rearrange("p (h d) -> p h d", h=H)
                )
                nc.sync.dma_start(v3[:, :, DH:VW], onesv[:, :, :])

            def emit_ypar_group(yfc):
                # wout partials over om chunks 0..KC-2; final chunk in phase 3
                pt = ps_sc.tile([P, N], F32, name="w", tag="w")
                for ih in range(2):
                    for kc in range(KC - 1):
                        nc.tensor.matmul(
                            pt[:, ih * NH:(ih + 1) * NH],
                            wo_sb[kc][:, yfc * P:(yfc + 1) * P],
                            om_sb[kc][:, ih * NH:(ih + 1) * NH],
                            start=(kc == 0), stop=(kc == KC - 2),
                        )
                nc.vector.tensor_copy(ypar_sb[yfc][:], pt[:])

            # ---------------- phase 2: per-head pipeline ----------------
            with (
                tc.tile_pool(name="el", bufs=8 * (LAG + 1)) as el_pool,
                tc.tile_pool(name="ework", bufs=4) as e_pool,
                tc.tile_pool(name="bc", bufs=LAG + 1) as bc_pool,
                tc.tile_pool(name="brs", bufs=2) as brs_pool,
                tc.tile_pool(name="ot", bufs=2) as ot_pool,
                tc.tile_pool(name="small", bufs=LAG + 1) as sm_pool,
                tc.tile_pool(name="dram", bufs=2, space="DRAM") as dr_pool,
            ):
                lid_hs = [lid_sb[h // 2][(h % 2) * DH:(h % 2) * DH + DH, :] for h in range(H)]
                q_hs = [qT_sb[h // 2][(h % 2) * DH:(h % 2) * DH + DH, :] for h in range(H)]
                k_hs = [kT_sb[h // 2][(h % 2) * DH:(h % 2) * DH + DH, :] for h in range(H)]
                st = {}
                # proj/wout groups spread across iterations (it -> {jc: arg})
                QK_SCHED = {0: {3: 0, 6: 4}, 1: {2: 1, 5: 5}, 2: {3: 2, 6: 6},
                            3: {3: 3, 6: 7}}
                YPAR_SCHED = {8: {6: 0}, 9: {2: 1, 5: 2}, 10: {2: 3}}
                for it in range(H + LAG + 1):
                    # stages: lidar(hl) | attention(ha) | finish(hf)
                    hl, ha, hf = it, it - LAG, it - LAG - 1

                    if 0 <= hf < H:
                        # finish: normalize O with the broadcast 1/s (single
                        # DVE mult reading PSUM -- frees the O bank), merge
                        sf = st[hf]
                        cf, offf = hf // 2, (hf % 2) * DH
                        ot_t = ot_pool.tile([DH, N], F16, name="ot", tag="ot")
                        nc.vector.tensor_mul(ot_t[:], sf["o"][0:DH, :], sf["brs"][:])
                        for ih in range(2):
                            mg = ps_sc.tile([P, NH], F32, name="w", tag="w")
                            nc.tensor.matmul(
                                mg[0:DH, 0:NH],
                                wm_sb[:],
                                ot_t[:, ih * NH:(ih + 1) * NH],
                                start=True, stop=True,
                            )
                            dst = om_sb[cf][offf:offf + DH, ih * NH:(ih + 1) * NH]
                            if need_bm:
                                nc.vector.tensor_scalar(
                                    out=dst, in0=mg[0:DH, 0:NH], scalar1=bm_sb[:],
                                    scalar2=None, op0=AX.add,
                                )
                            else:
                                nc.vector.tensor_copy(dst, mg[0:DH, 0:NH])
                        del st[hf]

                    if hl < H:
                        st[hl] = {
                            "explid": [el_pool.tile([P, N], F16, name="explid", tag="explid") for _ in range(TC)],
                            "slc": sm_pool.tile([P, TC], F32, name="slc", tag="slc"),
                        }
                    if 0 <= ha < H:
                        sa = st[ha]
                        sa["o"] = ps_o.tile([VW, N], F32, name="o", tag="o")
                        e_ts = {}

                    for jc in range(TC + 1):
                        fc = QK_SCHED.get(it, {}).get(jc)
                        if fc is not None:
                            emit_qk_group(fc)
                        if it == 1 and jc < TC:
                            emit_v_group(jc)
                        yfc = YPAR_SCHED.get(it, {}).get(jc)
                        if yfc is not None:
                            emit_ypar_group(yfc)

                        if hl < H and jc < TC:
                            # lidar scores + biased exp; row sums on DVE
                            # (symmetry of lsim makes them the softmax sums)
                            sl_ = st[hl]
                            pt = ps_sc.tile([P, N], F32, name="w", tag="w")
                            for ih in range(2):
                                nc.tensor.matmul(
                                    pt[:, ih * NH:(ih + 1) * NH],
                                    lid_hs[hl][:, jc * P:(jc + 1) * P],
                                    lid_hs[hl][:, ih * NH:(ih + 1) * NH],
                                    start=True, stop=True,
                                )
                            nc.scalar.activation(
                                sl_["explid"][jc][:], pt[:], EXP, bias=lb_sb[:],
                            )
                            nc.vector.tensor_reduce(
                                sl_["slc"][:, jc:jc + 1], sl_["explid"][jc][:],
                                mybir.AxisListType.X, AX.add,
                            )

                        if 0 <= ha < H and jc < TC:
                            # Lt = explid * bc (in place); mid = dots + Lt; E
                            expl = sa["explid"]
                            nc.vector.tensor_mul(expl[jc][:], expl[jc][:], sa["bc"][:])
                            mid = ps_sc.tile([P, N], F32, name="w", tag="w")
                            for ih in range(2):
                                nc.tensor.matmul(
                                    mid[:, ih * NH:(ih + 1) * NH],
                                    k_hs[ha][:, jc * P:(jc + 1) * P],
                                    q_hs[ha][:, ih * NH:(ih + 1) * NH],
                                    start=True, stop=False,
                                )
                                nc.tensor.matmul(
                                    mid[:, ih * NH:(ih + 1) * NH],
                                    id_sb[:],
                                    expl[jc][:, ih * NH:(ih + 1) * NH],
                                    start=False, stop=True,
                                )
                            e_t = e_pool.tile([P, N], F16, name="E", tag="E")
                            nc.scalar.activation(e_t[:], mid[:], EXP)
                            e_ts[jc] = e_t
                        if 0 <= ha < H and jc >= 1:
                            # vaug one step behind so PE never waits on exp
                            for ih in range(2):
                                nc.tensor.matmul(
                                    sa["o"][:, ih * NH:(ih + 1) * NH],
                                    v_sb[jc - 1][:, ha * VW:(ha + 1) * VW],
                                    e_ts[jc - 1][:, ih * NH:(ih + 1) * NH],
                                    start=(jc == 1), stop=(jc == TC),
                                )

                    if 0 <= ha < H:
                        # 1/s chain launched right after the last O matmul;
                        # brs is consumed by the finish stage next iteration
                        sa = st[ha]
                        rst = sm_pool.tile([1, N], F32, name="rst", tag="rst")
                        nc.vector.tensor_copy(rst[:], sa["o"][DH:VW, :])
                        s_d = dr_pool.tile([1, N], F32, name="s_d", tag="s_d")
                        nc.sync.dma_start(s_d[:], rst[:])
                        s2 = sm_pool.tile([TC, P], F32, name="s2", tag="s2")
                        nc.sync.dma_start(
                            s2[:], s_d[:].rearrange("o (q p) -> (o q) p", p=P)
                        )
                        nc.vector.reciprocal(s2[:], s2[:])
                        rs_d = dr_pool.tile([1, N], F32, name="rs_d", tag="rs_d")
                        nc.sync.dma_start(
                            rs_d[:].rearrange("o (q p) -> (o q) p", p=P), s2[:]
                        )
                        brs_t = brs_pool.tile([DH, N], F32, name="brs", tag="brs")
                        nc.gpsimd.dma_start(brs_t[:], rs_d[:].to_broadcast((DH, N)))
                        sa["brs"] = brs_t

                    if hl < H:
                        # bc chain: w1/sl column -> PE transpose -> DRAM
                        # bounce -> broadcast; consumed LAG iterations later
                        slc = st[hl]["slc"]
                        nc.vector.reciprocal(slc[:], slc[:])
                        nc.vector.tensor_scalar(
                            out=slc[:], in0=slc[:], scalar1=float(w1),
                            scalar2=None, op0=AX.mult,
                        )
                        rt = sm_pool.tile([TC, P], F16, name="rt", tag="rt")
                        tr = ps_sc.tile([P, NH], F32, name="w", tag="w")
                        nc.tensor.transpose(tr[0:TC, 0:P], slc[:], idf_sb[:])
                        nc.vector.tensor_copy(rt[:], tr[0:TC, 0:P])
                        rsl_d = dr_pool.tile([1, N], F16, name="rsl_d", tag="rsl_d")
                        nc.sync.dma_start(
                            rsl_d[:].rearrange("o (q p) -> (o q) p", p=P), rt[:]
                        )
                        bc_t = bc_pool.tile([P, N], F16, name="bc", tag="bc")
                        nc.sync.dma_start(
                            bc_t[0:64, :], rsl_d[:].to_broadcast((64, N))
                        )
                        nc.gpsimd.dma_start(
                            bc_t[64:P, :], rsl_d[:].to_broadcast((64, N))
                        )
                        st[hl]["bc"] = bc_t

            ctx_lp.__exit__(None, None, None)

            # ---------------- phase 3: final wout chunk + combine ----------------
            with tc.tile_pool(name="yout", bufs=2) as y_pool:
                for fc in range(KC):
                    pt = ps_sc.tile([P, N], F32, name="w", tag="w")
                    for ih in range(2):
                        nc.tensor.matmul(
                            pt[:, ih * NH:(ih + 1) * NH],
                            wo_sb[KC - 1][:, fc * P:(fc + 1) * P],
                            om_sb[KC - 1][:, ih * NH:(ih + 1) * NH],
                            start=True, stop=True,
                        )
                    yt = y_pool.tile([P, N], F32, name="yt", tag="yt")
                    nc.vector.tensor_add(yt[:], pt[:], ypar_sb[fc][:])
                    if need_bo:
                        nc.vector.tensor_scalar(
                            out=yt[:], in0=yt[:], scalar1=bo_sb[:, fc:fc + 1],
                            scalar2=None, op0=AX.add,
                        )
                    nc.sync.dma_start(y[fc * P:(fc + 1) * P, :], yt[:])

    nc.compile()
    return nc


def kernel(x, lidar, w_qkv, w_merge, b_merge, w_out, b_out, conv_w, conv_b, **_):
    x = np.asarray(x, np.float32)
    lidar = np.asarray(lidar, np.float32)
    w_qkv = np.asarray(w_qkv, np.float32)
    w_merge = np.asarray(w_merge, np.float32)
    b_merge = np.asarray(b_merge, np.float32)
    w_out = np.asarray(w_out, np.float32)
    b_out = np.asarray(b_out, np.float32)
    w0, w1 = float(np.asarray(conv_w)[0]), float(np.asarray(conv_w)[1])

    need_bm = bool(np.any(b_merge != 0))
    need_bo = bool(np.any(b_out != 0))
    key = (round(w1, 12), need_bm, need_bo)
    if key not in _cache:
        _cache.clear()
        _cache[key] = _build(w1, need_bm, need_bo)
    nc = _cache[key]

    # host-side weight prep: transposes + constant folds + fp16 casts
    wqkT = np.ascontiguousarray(w_qkv[0:QK].T)       # [512 dim, 1024 q|k feats]
    wqkT[:, 0:INNER] *= np.float32(SCALE * w0)       # fold w0*SCALE into q
    wqkT = wqkT.astype(np.float16)
    wvT = np.ascontiguousarray(w_qkv[QK:3 * INNER].T).astype(np.float16)
    wmT = np.ascontiguousarray(w_merge.T).astype(np.float16)
    woT = np.ascontiguousarray(w_out.T).astype(np.float16)
    identity = np.eye(P, dtype=np.float16)
    identityf = np.eye(P, dtype=np.float32)
    bm_c = np.ascontiguousarray(b_merge.reshape(DH, 1))
    bo_c = np.ascontiguousarray(b_out.reshape(KC, P).T)

    sqrt_scale = np.float32(SCALE ** 0.5)
    in_maps = []
    for b in range(B):
        in_maps.append({
            "xT": np.ascontiguousarray(x[b].T).astype(np.float16),
            "lidT": (lidar[b].T * sqrt_scale).astype(np.float16),
            "wqkT": wqkT,
            "wvT": wvT,
            "wmT": wmT,
            "woT": woT,
            "ident": identity,
            "identf": identityf,
            "onesv": np.ones((P, H, 1), np.float16),
            "bm": bm_c,
            "bo": bo_c,
        })

    try:
        res = run_bass_kernel_spmd(nc, in_maps, core_ids=list(range(B)))
    except Exception:
        # transient NRT device wedges recover on a fresh attempt
        import time as _time

        _time.sleep(5)
        res = run_bass_kernel_spmd(nc, in_maps, core_ids=list(range(B)))
    kernel._last_results = res

    out = np.stack([res.results[b]["y"].T for b in range(B)])
    return (out, lidar)


# revision 8
# speedup vs baseline: 1.4420x; 1.4420x over previous
"""Fused dual-softmax attention (nn_Attention sparse_attention) on 8x TRN2.

Sharding: data-parallel over batch -- one batch element per NeuronCore.

Per-core pipeline (feature-major activations, key-major score matrices).
All matmul operands are fp16 (1 cyc/row on PE); PSUM accumulation is fp32.

Four-stage software pipeline over heads, one head per iteration, with the
lidar stage running TWO iterations ahead of the attention stage so the
per-head DMA bounce chains (bc broadcast for the lidar-softmax column
scale, 1/s broadcast for the final softmax normalize) never sit on the
critical path.  This keeps ScalarE (the exp engine, the true roofline of
this kernel: 128 exp passes over [128,1024] tiles ~ 147us) nearly 100%
busy and keeps the PE free of the >1us stalls that caused HAM clock
oscillation (K=4/8 throttling) in earlier revisions.

  qT/kT = w_{q,k} @ x.T            (w0*SCALE pre-folded into wq)
  v     = x @ w_v.T  token-major, augmented with a ones column per head
  iteration it (hl=it lidar, ha=it-2 attention, hf=it-3 finish):
    F(hf):  ot = O_psum[0:64] * brs    (single DVE mult reading PSUM,
                                        frees the O bank) -> merge -> om
    L(hl):  lsim = lid_h.T @ lid_h     (symmetric; sqrt(SCALE) folded)
            explid = exp(lsim - 8)     (ACT; bias keeps fp16 range; shift
                                        cancels in the lidar softmax)
            sl[:,jc] = rowsum(explid)  (DVE X-reduce; symmetry => row sums
                                        equal the key-axis softmax sums)
    A(ha):  Lt = explid * bc           (DVE fp16 2x, in place)
            mid = k_h.T q_h + I @ Lt   (dots MM + identity MM into PSUM)
            E  = exp(mid)              (ACT, PSUM -> SBUF fp16)
            O += [v_h | 1].T @ E       (PE; row 64 = softmax denominators)
            then start 1/s chain: copy s row, bounce to [8,128], recip,
            bounce back, broadcast -> brs (consumed by F next iteration)
    L-tail: bc chain: recip(sl)*w1 -> PE transpose -> DRAM bounce ->
            broadcast bc_t (consumed by A two iterations later)
  outT = w_out.T.T @ om (+ b_out) -> DMA out; host transposes back.

Softmax max-subtraction is dropped (|scores| <= ~20, exp safe) and conv_b
is dropped (softmax is shift-invariant along the reduced axis).
"""

import sys

try:
    import concourse.bass as bass
except ImportError:  # pragma: no cover
    sys.path.insert(0, "/opt/trn_rl_repo")
    import concourse.bass as bass

import numpy as np

import concourse.mybir as mybir
from concourse import bacc
from concourse.tile import TileContext
from concourse.bass_utils import run_bass_kernel_spmd

F32 = mybir.dt.float32
F16 = mybir.dt.float16
AX = mybir.AluOpType
EXP = mybir.ActivationFunctionType.Exp

B, N, DIM, H, DH = 8, 1024, 512, 8, 64
INNER = H * DH          # 512
QK = 2 * INNER          # 1024 (q|k feature rows of w_qkv)
SCALE = DH ** -0.5
LBIAS = -8.0            # lidar-exp shift: keeps exp(lsim) inside fp16 range
P = 128
NH = N // 2             # 512: max matmul free dim / fp32 PSUM bank
KC = DIM // P           # 4 contraction chunks
TC = N // P             # 8 token chunks
VW = DH + 1             # per-head v width incl. ones column
LAG = 2                 # lidar stage runs LAG iterations ahead of attention

_cache = {}


def _build(w1, need_bm, need_bo):
    nc = bacc.Bacc("TRN2", target_bir_lowering=False, debug=False, num_devices=B)

    xT = nc.dram_tensor("xT", [DIM, N], F16, kind="ExternalInput")
    lidT = nc.dram_tensor("lidT", [DIM, N], F16, kind="ExternalInput")
    wqkT = nc.dram_tensor("wqkT", [DIM, QK], F16, kind="ExternalInput")
    wvT = nc.dram_tensor("wvT", [DIM, INNER], F16, kind="ExternalInput")
    wmT = nc.dram_tensor("wmT", [DH, DH], F16, kind="ExternalInput")
    woT = nc.dram_tensor("woT", [INNER, DIM], F16, kind="ExternalInput")
    ident = nc.dram_tensor("ident", [P, P], F16, kind="ExternalInput")
    identf = nc.dram_tensor("identf", [P, P], F32, kind="ExternalInput")
    onesv = nc.dram_tensor("onesv", [P, H, 1], F16, kind="ExternalInput")
    bm = nc.dram_tensor("bm", [DH, 1], F32, kind="ExternalInput")
    bo = nc.dram_tensor("bo", [P, KC], F32, kind="ExternalInput")
    y = nc.dram_tensor("y", [DIM, N], F32, kind="ExternalOutput")

    with TileContext(nc) as tc:
        with (
            tc.tile_pool(name="persist", bufs=1) as pp,
            tc.tile_pool(name="ps_sc", bufs=2, space="PSUM") as ps_sc,
            tc.tile_pool(name="ps_o", bufs=2, space="PSUM") as ps_o,
        ):
            # ---------------- persistent SBUF ----------------
            lid_sb = [pp.tile([P, N], F16, name=f"lid{i}", tag=f"lid{i}") for i in range(KC)]
            qT_sb = [pp.tile([P, N], F16, name=f"qT{i}", tag=f"qT{i}") for i in range(KC)]
            kT_sb = [pp.tile([P, N], F16, name=f"kT{i}", tag=f"kT{i}") for i in range(KC)]
            v_sb = [pp.tile([P, H * VW], F16, name=f"v{i}", tag=f"v{i}") for i in range(TC)]
            om_sb = [pp.tile([P, N], F16, name=f"om{i}", tag=f"om{i}") for i in range(KC)]
            id_sb = pp.tile([P, P], F16, name="ident", tag="ident")
            idf_sb = pp.tile([P, P], F32, name="identf", tag="identf")
            wm_sb = pp.tile([DH, DH], F16, name="wm", tag="wm")
            bm_sb = pp.tile([DH, 1], F32, name="bm", tag="bm")
            bo_sb = pp.tile([P, KC], F32, name="bo", tag="bo")
            lb_sb = pp.tile([P, 1], F32, name="lb", tag="lb")
            nc.vector.memset(lb_sb[:], LBIAS)
            wo_sb = [pp.tile([P, DIM], F16, name=f"wo{i}", tag=f"wo{i}") for i in range(KC)]
            ypar_sb = [pp.tile([P, N], F32, name=f"ypar{i}", tag=f"ypar{i}") for i in range(KC)]

            for c in range(KC):
                nc.sync.dma_start(lid_sb[c][:], lidT[c * P:(c + 1) * P, :])
            nc.sync.dma_start(id_sb[:], ident[:, :])
            nc.sync.dma_start(idf_sb[:], identf[:, :])
            nc.sync.dma_start(wm_sb[:], wmT[:, :])
            nc.sync.dma_start(bm_sb[:], bm[:, :])
            nc.sync.dma_start(bo_sb[:], bo[:, :])
            for kc in range(KC):
                nc.sync.dma_start(wo_sb[kc][:], woT[kc * P:(kc + 1) * P, :])

            # phase-1 loads (projection matmuls interleave into the early
            # pipeline iterations below to keep PE density high)
            lp = ctx_lp = tc.tile_pool(name="load", bufs=1)
            lp = ctx_lp.__enter__()
            x_sb = [lp.tile([P, N], F16, name=f"x{i}", tag=f"x{i}") for i in range(KC)]
            wqk_sb = [lp.tile([P, QK], F16, name=f"wqk{i}", tag=f"wqk{i}") for i in range(KC)]
            wv_sb = [lp.tile([P, INNER], F16, name=f"wv{i}", tag=f"wv{i}") for i in range(KC)]
            for c in range(KC):
                nc.sync.dma_start(x_sb[c][:], xT[c * P:(c + 1) * P, :])
                nc.sync.dma_start(wqk_sb[c][:], wqkT[c * P:(c + 1) * P, :])
                nc.sync.dma_start(wv_sb[c][:], wvT[c * P:(c + 1) * P, :])

            def emit_qk_group(fc):
                # qT|kT feature-major: out[fc,:] = sum_kc wqk[kc,fc].T @ xT[kc,:]
                dst = (qT_sb if fc < KC else kT_sb)[fc % KC]
                for ih in range(2):
                    pt = ps_sc.tile([P, NH], F32, name="w", tag="w")
                    for kc in range(KC):
                        nc.tensor.matmul(
                            pt[:],
                            wqk_sb[kc][:, fc * P:(fc + 1) * P],
                            x_sb[kc][:, ih * NH:(ih + 1) * NH],
                            start=(kc == 0), stop=(kc == KC - 1),
                        )
                    nc.vector.tensor_copy(dst[:, ih * NH:(ih + 1) * NH], pt[:])

            def emit_v_group(t):
                # v token-major: v[t,:] = sum_kc xT[kc,t].T @ wvT[kc,:]
                pt = ps_sc.tile([P, INNER], F32, name="w", tag="w")
                for kc in range(KC):
                    nc.tensor.matmul(
                        pt[:],
                        x_sb[kc][:, t * P:(t + 1) * P],
                        wv_sb[kc][:],
                        start=(kc == 0), stop=(kc == KC - 1),
                    )
                v3 = v_sb[t][:].rearrange("p (h w) -> p h w", h=H)
                nc.vector.tensor_copy(
                    v3[:, :, 0:DH], pt[:].rearrange("p (h d) -> p h d", h=H)
                )
                nc.sync.dma_start(v3[:, :, DH:VW], onesv[:, :, :])

            def emit_ypar_group(yfc):
                # wout partials over om chunks 0..KC-2; final chunk in phase 3
                pt = ps_sc.tile([P, N], F32, name="w", tag="w")
                for ih in range(2):
                    for kc in range(KC - 1):
                        nc.tensor.matmul(
                            pt[:, ih * NH:(ih + 1) * NH],
                            wo_sb[kc][:, yfc * P:(yfc + 1) * P],
                            om_sb[kc][:, ih * NH:(ih + 1) * NH],
                            start=(kc == 0), stop=(kc == KC - 2),
                        )
                nc.vector.tensor_copy(ypar_sb[yfc][:], pt[:])

            # ---------------- phase 2: per-head pipeline ----------------
            with (
                tc.tile_pool(name="el", bufs=8 * (LAG + 1)) as el_pool,
                tc.tile_pool(name="ework", bufs=4) as e_pool,
                tc.tile_pool(name="bc", bufs=LAG + 1) as bc_pool,
                tc.tile_pool(name="brs", bufs=2) as brs_pool,
                tc.tile_pool(name="ot", bufs=2) as ot_pool,
                tc.tile_pool(name="small", bufs=LAG + 1) as sm_pool,
                tc.tile_pool(name="dram", bufs=2, space="DRAM") as dr_pool,
            ):
                lid_hs = [lid_sb[h // 2][(h % 2) * DH:(h % 2) * DH + DH, :] for h in range(H)]
                q_hs = [qT_sb[h // 2][(h % 2) * DH:(h % 2) * DH + DH, :] for h in range(H)]
                k_hs = [kT_sb[h // 2][(h % 2) * DH:(h % 2) * DH + DH, :] for h in range(H)]
                st = {}
                # proj/wout groups spread across iterations (it -> {jc: arg})
                QK_SCHED = {0: {3: 0, 6: 4}, 1: {2: 1, 5: 5}, 2: {3: 2, 6: 6},
                            3: {3: 3, 6: 7}}
                YPAR_SCHED = {9: {2: 0, 6: 1}, 10: {2: 2, 6: 3}}
                for it in range(H + LAG + 1):
                    # stages: lidar(hl) | attention(ha) | finish(hf)
                    hl, ha, hf = it, it - LAG, it - LAG - 1

                    if hl < H:
                        st[hl] = {
                            "explid": [el_pool.tile([P, N], F16, name="explid", tag="explid") for _ in range(TC)],
                            "slc": sm_pool.tile([P, TC], F32, name="slc", tag="slc"),
                        }
                    if 0 <= ha < H:
                        sa = st[ha]
                        sa["o"] = ps_o.tile([VW, N], F32, name="o", tag="o")
                        e_ts = {}

                    for jc in range(TC + 1):
                        fc = QK_SCHED.get(it, {}).get(jc)
                        if fc is not None:
                            emit_qk_group(fc)
                        if it == 1 and jc < TC:
                            emit_v_group(jc)
                        yfc = YPAR_SCHED.get(it, {}).get(jc)
                        if yfc is not None:
                            emit_ypar_group(yfc)

                        if 0 <= ha < H and jc < TC:
                            # Lt = explid * bc (in place); mid = dots + Lt; E
                            expl = sa["explid"]
                            nc.vector.tensor_mul(expl[jc][:], expl[jc][:], sa["bc"][:])
                            mid = ps_sc.tile([P, N], F32, name="w", tag="w")
                            for ih in range(2):
                                nc.tensor.matmul(
                                    mid[:, ih * NH:(ih + 1) * NH],
                                    k_hs[ha][:, jc * P:(jc + 1) * P],
                                    q_hs[ha][:, ih * NH:(ih + 1) * NH],
                                    start=True, stop=False,
                                )
                                nc.tensor.matmul(
                                    mid[:, ih * NH:(ih + 1) * NH],
                                    id_sb[:],
                                    expl[jc][:, ih * NH:(ih + 1) * NH],
                                    start=False, stop=True,
                                )
                            e_t = e_pool.tile([P, N], F16, name="E", tag="E")
                            nc.scalar.activation(e_t[:], mid[:], EXP)
                            e_ts[jc] = e_t
                        if 0 <= ha < H and jc >= 1:
                            # vaug one step behind so PE never waits on exp
                            for ih in range(2):
                                nc.tensor.matmul(
                                    sa["o"][:, ih * NH:(ih + 1) * NH],
                                    v_sb[jc - 1][:, ha * VW:(ha + 1) * VW],
                                    e_ts[jc - 1][:, ih * NH:(ih + 1) * NH],
                                    start=(jc == 1), stop=(jc == TC),
                                )

                        if hl < H and jc < TC:
                            # lidar scores + biased exp w/ accumulate (sl rows
                            # via symmetry of lsim)
                            sl_ = st[hl]
                            pt = ps_sc.tile([P, N], F32, name="w", tag="w")
                            for ih in range(2):
                                nc.tensor.matmul(
                                    pt[:, ih * NH:(ih + 1) * NH],
                                    lid_hs[hl][:, jc * P:(jc + 1) * P],
                                    lid_hs[hl][:, ih * NH:(ih + 1) * NH],
                                    start=True, stop=True,
                                )
                            nc.scalar.activation(
                                sl_["explid"][jc][:], pt[:], EXP, bias=lb_sb[:],
                                accum_out=sl_["slc"][:, jc:jc + 1],
                            )

                    if 0 <= hf < H:
                        # finish: normalize O with the broadcast 1/s (single
                        # DVE mult reading PSUM -- frees the O bank), merge;
                        # emitted late so the 1/s chain has ~1.5 iterations
                        sf = st[hf]
                        cf, offf = hf // 2, (hf % 2) * DH
                        ot_t = ot_pool.tile([DH, N], F16, name="ot", tag="ot")
                        nc.vector.tensor_mul(ot_t[:], sf["o"][0:DH, :], sf["brs"][:])
                        for ih in range(2):
                            mg = ps_sc.tile([P, NH], F32, name="w", tag="w")
                            nc.tensor.matmul(
                                mg[0:DH, 0:NH],
                                wm_sb[:],
                                ot_t[:, ih * NH:(ih + 1) * NH],
                                start=True, stop=True,
                            )
                            dst = om_sb[cf][offf:offf + DH, ih * NH:(ih + 1) * NH]
                            if need_bm:
                                nc.vector.tensor_scalar(
                                    out=dst, in0=mg[0:DH, 0:NH], scalar1=bm_sb[:],
                                    scalar2=None, op0=AX.add,
                                )
                            else:
                                nc.vector.tensor_copy(dst, mg[0:DH, 0:NH])
                        del st[hf]

                    if 0 <= ha < H:
                        # 1/s chain launched right after the last O matmul;
                        # brs is consumed by the finish stage next iteration
                        sa = st[ha]
                        rst = sm_pool.tile([1, N], F32, name="rst", tag="rst")
                        nc.vector.tensor_copy(rst[:], sa["o"][DH:VW, :])
                        s_d = dr_pool.tile([1, N], F32, name="s_d", tag="s_d")
                        nc.sync.dma_start(s_d[:], rst[:])
                        s2 = sm_pool.tile([TC, P], F32, name="s2", tag="s2")
                        nc.sync.dma_start(
                            s2[:], s_d[:].rearrange("o (q p) -> (o q) p", p=P)
                        )
                        nc.vector.reciprocal(s2[:], s2[:])
                        rs_d = dr_pool.tile([1, N], F32, name="rs_d", tag="rs_d")
                        nc.sync.dma_start(
                            rs_d[:].rearrange("o (q p) -> (o q) p", p=P), s2[:]
                        )
                        brs_t = brs_pool.tile([DH, N], F32, name="brs", tag="brs")
                        nc.gpsimd.dma_start(brs_t[:], rs_d[:].to_broadcast((DH, N)))
                        sa["brs"] = brs_t

                    if hl < H:
                        # bc chain: w1/sl column -> PE transpose -> DRAM
                        # bounce -> broadcast; consumed LAG iterations later
                        slc = st[hl]["slc"]
                        nc.vector.reciprocal(slc[:], slc[:])
                        nc.vector.tensor_scalar(
                            out=slc[:], in0=slc[:], scalar1=float(w1),
                            scalar2=None, op0=AX.mult,
                        )
                        rt = sm_pool.tile([TC, P], F16, name="rt", tag="rt")
                        tr = ps_sc.tile([P, NH], F32, name="w", tag="w")
                        nc.tensor.transpose(tr[0:TC, 0:P], slc[:], idf_sb[:])
                        nc.vector.tensor_copy(rt[:], tr[0:TC, 0:P])
                        rsl_d = dr_pool.tile([1, N], F16, name="rsl_d", tag="rsl_d")
                        nc.sync.dma_start(
                            rsl_d[:].rearrange("o (q p) -> (o q) p", p=P), rt[:]
                        )
                        bc_t = bc_pool.tile([P, N], F16, name="bc", tag="bc")
                        nc.sync.dma_start(
                            bc_t[0:64, :], rsl_d[:].to_broadcast((64, N))
                        )
                        nc.gpsimd.dma_start(
                            bc_t[64:P, :], rsl_d[:].to_broadcast((64, N))
                        )
                        st[hl]["bc"] = bc_t

            ctx_lp.__exit__(None, None, None)

            # ---------------- phase 3: final wout chunk + combine ----------------
            with tc.tile_pool(name="yout", bufs=2) as y_pool:
                for fc in range(KC):
                    pt = ps_sc.tile([P, N], F32, name="w", tag="w")
                    for ih in range(2):
                        nc.tensor.matmul(
                            pt[:, ih * NH:(ih + 1) * NH],
                            wo_sb[KC - 1][:, fc * P:(fc + 1) * P],
                            om_sb[KC - 1][:, ih * NH:(ih + 1) * NH],
                            start=True, stop=True,
                        )
                    yt = y_pool.tile([P, N], F32, name="yt", tag="yt")
                    nc.vector.tensor_add(yt[:], pt[:], ypar_sb[fc][:])
                    if need_bo:
                        nc.vector.tensor_scalar(
                            out=yt[:], in0=yt[:], scalar1=bo_sb[:, fc:fc + 1],
                            scalar2=None, op0=AX.add,
                        )
                    nc.sync.dma_start(y[fc * P:(fc + 1) * P, :], yt[:])

    nc.compile()
    return nc


def kernel(x, lidar, w_qkv, w_merge, b_merge, w_out, b_out, conv_w, conv_b, **_):
    x = np.asarray(x, np.float32)
    lidar = np.asarray(lidar, np.float32)
    w_qkv = np.asarray(w_qkv, np.float32)
    w_merge = np.asarray(w_merge, np.float32)
    b_merge = np.asarray(b_merge, np.float32)
    w_out = np.asarray(w_out, np.float32)
    b_out = np.asarray(b_out, np.float32)
    w0, w1 = float(np.asarray(conv_w)[0]), float(np.asarray(conv_w)[1])

    need_bm = bool(np.any(b_merge != 0))
    need_bo = bool(np.any(b_out != 0))
    key = (round(w1, 12), need_bm, need_bo)
    if key not in _cache:
        _cache.clear()
        _cache[key] = _build(w1, need_bm, need_bo)
    nc = _cache[key]

    # host-side weight prep: transposes + constant folds + fp16 casts
    wqkT = np.ascontiguousarray(w_qkv[0:QK].T)       # [512 dim, 1024 q|k feats]
    wqkT[:, 0:INNER] *= np.float32(SCALE * w0)       # fold w0*SCALE into q
    wqkT = wqkT.astype(np.float16)
    wvT = np.ascontiguousarray(w_qkv[QK:3 * INNER].T).astype(np.float16)
    wmT = np.ascontiguousarray(w_merge.T).astype(np.float16)
    woT = np.ascontiguousarray(w_out.T).astype(np.float16)
    identity = np.eye(P, dtype=np.float16)
    identityf = np.eye(P, dtype=np.float32)
    bm_c = np.ascontiguousarray(b_merge.reshape(DH, 1))
    bo_c = np.ascontiguousarray(b_out.reshape(KC, P).T)

    sqrt_scale = np.float32(SCALE ** 0.5)
    in_maps = []
    for b in range(B):
        in_maps.append({
            "xT": np.ascontiguousarray(x[b].T).astype(np.float16),
            "lidT": (lidar[b].T * sqrt_scale).astype(np.float16),
            "wqkT": wqkT,
            "wvT": wvT,
            "wmT": wmT,
            "woT": woT,
            "ident": identity,
            "identf": identityf,
            "onesv": np.ones((P, H, 1), np.float16),
            "bm": bm_c,
            "bo": bo_c,
        })

    try:
        res = run_bass_kernel_spmd(nc, in_maps, core_ids=list(range(B)))
    except Exception:
        # transient NRT device wedges recover on a fresh attempt
        import time as _time

        _time.sleep(5)
        res = run_bass_kernel_spmd(nc, in_maps, core_ids=list(range(B)))
    kernel._last_results = res

    out = np.stack([res.results[b]["y"].T for b in range(B)])
    return (out, lidar)


# revision 13
# speedup vs baseline: 1.6482x; 1.1430x over previous
"""Fused dual-softmax attention (nn_Attention sparse_attention) on 8x TRN2.

Sharding: data-parallel over batch -- one batch element per NeuronCore.

Per-core pipeline (feature-major activations, key-major score matrices).
All matmul operands are fp16 (1 cyc/row on PE); PSUM accumulation is fp32.

Four-stage software pipeline over heads, one head per iteration, with the
lidar stage running TWO iterations ahead of the attention stage so the
per-head DMA bounce chains (bc broadcast for the lidar-softmax column
scale, 1/s broadcast for the final softmax normalize) never sit on the
critical path.  This keeps ScalarE (the exp engine, the true roofline of
this kernel: 128 exp passes over [128,1024] tiles ~ 147us) nearly 100%
busy and keeps the PE free of the >1us stalls that caused HAM clock
oscillation (K=4/8 throttling) in earlier revisions.

  qT/kT = w_{q,k} @ x.T            (w0*SCALE pre-folded into wq)
  v     = x @ w_v.T  token-major, augmented with a ones column per head
  iteration it (hl=it lidar, ha=it-2 attention, hf=it-3 finish):
    F(hf):  ot = O_psum[0:64] * brs    (single DVE mult reading PSUM,
                                        frees the O bank) -> merge -> om
    L(hl):  lsim = lid_h.T @ lid_h     (symmetric; sqrt(SCALE) folded)
            explid = exp(lsim - 8)     (ACT; bias keeps fp16 range; shift
                                        cancels in the lidar softmax)
            sl[:,jc] = rowsum(explid)  (DVE X-reduce; symmetry => row sums
                                        equal the key-axis softmax sums)
    A(ha):  Lt = explid * bc           (DVE fp16 2x, in place)
            mid = k_h.T q_h + I @ Lt   (dots MM + identity MM into PSUM)
            E  = exp(mid)              (ACT, PSUM -> SBUF fp16)
            O += [v_h | 1].T @ E       (PE; row 64 = softmax denominators)
            then start 1/s chain: copy s row, bounce to [8,128], recip,
            bounce back, broadcast -> brs (consumed by F next iteration)
    L-tail: bc chain: recip(sl)*w1 -> PE transpose -> DRAM bounce ->
            broadcast bc_t (consumed by A two iterations later)
  outT = w_out.T.T @ om (+ b_out) -> DMA out; host transposes back.

Softmax max-subtraction is dropped (|scores| <= ~20, exp safe) and conv_b
is dropped (softmax is shift-invariant along the reduced axis).
"""

import sys

try:
    import concourse.bass as bass
except ImportError:  # pragma: no cover
    sys.path.insert(0, "/opt/trn_rl_repo")
    import concourse.bass as bass

import numpy as np

import concourse.mybir as mybir
from concourse import bacc
from concourse.tile import TileContext
from concourse.bass_utils import run_bass_kernel_spmd

F32 = mybir.dt.float32
F16 = mybir.dt.float16
AX = mybir.AluOpType
EXP = mybir.ActivationFunctionType.Exp

B, N, DIM, H, DH = 8, 1024, 512, 8, 64
INNER = H * DH          # 512
QK = 2 * INNER          # 1024 (q|k feature rows of w_qkv)
SCALE = DH ** -0.5
LBIAS = -8.0            # lidar-exp shift: keeps exp(lsim) inside fp16 range
P = 128
NH = N // 2             # 512: max matmul free dim / fp32 PSUM bank
KC = DIM // P           # 4 contraction chunks
TC = N // P             # 8 token chunks
VW = DH + 1             # per-head v width incl. ones column
LAG = 2                 # lidar stage runs LAG iterations ahead of attention

_cache = {}


def _build(w1, need_bm, need_bo):
    nc = bacc.Bacc("TRN2", target_bir_lowering=False, debug=False, num_devices=B)

    xT = nc.dram_tensor("xT", [DIM, N], F16, kind="ExternalInput")
    lidT = nc.dram_tensor("lidT", [DIM, N], F16, kind="ExternalInput")
    wqkT = nc.dram_tensor("wqkT", [DIM, QK], F16, kind="ExternalInput")
    wvT = nc.dram_tensor("wvT", [DIM, INNER], F16, kind="ExternalInput")
    wmT = nc.dram_tensor("wmT", [DH, DH], F16, kind="ExternalInput")
    woT = nc.dram_tensor("woT", [INNER, DIM], F16, kind="ExternalInput")
    ident = nc.dram_tensor("ident", [P, P], F16, kind="ExternalInput")
    identf = nc.dram_tensor("identf", [P, P], F32, kind="ExternalInput")
    onesv = nc.dram_tensor("onesv", [P, H, 1], F16, kind="ExternalInput")
    bm = nc.dram_tensor("bm", [DH, 1], F32, kind="ExternalInput")
    bo = nc.dram_tensor("bo", [P, KC], F32, kind="ExternalInput")
    y = nc.dram_tensor("y", [DIM, N], F32, kind="ExternalOutput")

    with TileContext(nc) as tc:
        with (
            tc.tile_pool(name="persist", bufs=1) as pp,
            tc.tile_pool(name="ps_sc", bufs=3, space="PSUM") as ps_sc,
            tc.tile_pool(name="ps_o", bufs=1, space="PSUM") as ps_o,
        ):
            # ---------------- persistent SBUF ----------------
            lid_sb = [pp.tile([P, N], F16, name=f"lid{i}", tag=f"lid{i}") for i in range(KC)]
            qT_sb = [pp.tile([P, N], F16, name=f"qT{i}", tag=f"qT{i}") for i in range(KC)]
            kT_sb = [pp.tile([P, N], F16, name=f"kT{i}", tag=f"kT{i}") for i in range(KC)]
            v_sb = [pp.tile([P, H * VW], F16, name=f"v{i}", tag=f"v{i}") for i in range(TC)]
            om_sb = [pp.tile([P, N], F16, name=f"om{i}", tag=f"om{i}") for i in range(KC)]
            id_sb = pp.tile([P, P], F16, name="ident", tag="ident")
            idf_sb = pp.tile([P, P], F32, name="identf", tag="identf")
            wm_sb = pp.tile([DH, DH], F16, name="wm", tag="wm")
            bm_sb = pp.tile([DH, 1], F32, name="bm", tag="bm")
            bo_sb = pp.tile([P, KC], F32, name="bo", tag="bo")
            lb_sb = pp.tile([P, 1], F32, name="lb", tag="lb")
            nc.vector.memset(lb_sb[:], LBIAS)
            wo_sb = [pp.tile([P, DIM], F16, name=f"wo{i}", tag=f"wo{i}") for i in range(KC)]
            ypar_sb = [pp.tile([P, N], F32, name=f"ypar{i}", tag=f"ypar{i}") for i in range(KC)]

            for c in range(KC):
                nc.sync.dma_start(lid_sb[c][:], lidT[c * P:(c + 1) * P, :])
            nc.sync.dma_start(id_sb[:], ident[:, :])
            nc.sync.dma_start(idf_sb[:], identf[:, :])
            nc.sync.dma_start(wm_sb[:], wmT[:, :])
            nc.sync.dma_start(bm_sb[:], bm[:, :])
            nc.sync.dma_start(bo_sb[:], bo[:, :])
            for kc in range(KC):
                nc.sync.dma_start(wo_sb[kc][:], woT[kc * P:(kc + 1) * P, :])

            # phase-1 loads (projection matmuls interleave into the early
            # pipeline iterations below to keep PE density high)
            lp = ctx_lp = tc.tile_pool(name="load", bufs=1)
            lp = ctx_lp.__enter__()
            x_sb = [lp.tile([P, N], F16, name=f"x{i}", tag=f"x{i}") for i in range(KC)]
            wqk_sb = [lp.tile([P, QK], F16, name=f"wqk{i}", tag=f"wqk{i}") for i in range(KC)]
            wv_sb = [lp.tile([P, INNER], F16, name=f"wv{i}", tag=f"wv{i}") for i in range(KC)]
            for c in range(KC):
                nc.sync.dma_start(x_sb[c][:], xT[c * P:(c + 1) * P, :])
                nc.sync.dma_start(wqk_sb[c][:], wqkT[c * P:(c + 1) * P, :])
                nc.sync.dma_start(wv_sb[c][:], wvT[c * P:(c + 1) * P, :])

            def emit_qk_group(fc):
                # qT|kT feature-major: out[fc,:] = sum_kc wqk[kc,fc].T @ xT[kc,:]
                dst = (qT_sb if fc < KC else kT_sb)[fc % KC]
                for ih in range(2):
                    pt = ps_sc.tile([P, NH], F32, name="w", tag="w")
                    for kc in range(KC):
                        nc.tensor.matmul(
                            pt[:],
                            wqk_sb[kc][:, fc * P:(fc + 1) * P],
                            x_sb[kc][:, ih * NH:(ih + 1) * NH],
                            start=(kc == 0), stop=(kc == KC - 1),
                        )
                    nc.vector.tensor_copy(dst[:, ih * NH:(ih + 1) * NH], pt[:])

            def emit_v_group(t):
                # v token-major: v[t,:] = sum_kc xT[kc,t].T @ wvT[kc,:]
                pt = ps_sc.tile([P, INNER], F32, name="w", tag="w")
                for kc in range(KC):
                    nc.tensor.matmul(
                        pt[:],
                        x_sb[kc][:, t * P:(t + 1) * P],
                        wv_sb[kc][:],
                        start=(kc == 0), stop=(kc == KC - 1),
                    )
                v3 = v_sb[t][:].rearrange("p (h w) -> p h w", h=H)
                nc.vector.tensor_copy(
                    v3[:, :, 0:DH], pt[:].rearrange("p (h d) -> p h d", h=H)
                )
                nc.sync.dma_start(v3[:, :, DH:VW], onesv[:, :, :])

            def emit_ypar_group(yfc):
                # wout partials over om chunks 0..KC-2; final chunk in phase 3
                pt = ps_sc.tile([P, N], F32, name="w", tag="w")
                for ih in range(2):
                    for kc in range(KC - 1):
                        nc.tensor.matmul(
                            pt[:, ih * NH:(ih + 1) * NH],
                            wo_sb[kc][:, yfc * P:(yfc + 1) * P],
                            om_sb[kc][:, ih * NH:(ih + 1) * NH],
                            start=(kc == 0), stop=(kc == KC - 2),
                        )
                nc.vector.tensor_copy(ypar_sb[yfc][:], pt[:])

            # ---------------- phase 2: per-head pipeline ----------------
            with (
                tc.tile_pool(name="el", bufs=8 * (LAG + 1)) as el_pool,
                tc.tile_pool(name="ework", bufs=4) as e_pool,
                tc.tile_pool(name="bc", bufs=LAG + 1) as bc_pool,
                tc.tile_pool(name="brs", bufs=2) as brs_pool,
                tc.tile_pool(name="ot", bufs=2) as ot_pool,
                tc.tile_pool(name="small", bufs=LAG + 1) as sm_pool,
                tc.tile_pool(name="dram", bufs=2, space="DRAM") as dr_pool,
            ):
                lid_hs = [lid_sb[h // 2][(h % 2) * DH:(h % 2) * DH + DH, :] for h in range(H)]
                q_hs = [qT_sb[h // 2][(h % 2) * DH:(h % 2) * DH + DH, :] for h in range(H)]
                k_hs = [kT_sb[h // 2][(h % 2) * DH:(h % 2) * DH + DH, :] for h in range(H)]
                st = {}
                # proj/wout groups spread one per iteration as steady PE
                # filler (keeps HAM warm); deadlines: chunk c by iter 2c+2
                QK_SCHED = {0: {3: 0}, 1: {3: 4}, 2: {3: 1}, 3: {3: 5},
                            4: {3: 2}, 5: {3: 6}, 6: {3: 3}, 7: {3: 7}}
                YPAR_SCHED = {9: {2: 0, 6: 1}, 10: {2: 2, 6: 3}}
                for it in range(H + LAG + 1):
                    # stages: lidar(hl) | attention(ha) | finish(hf)
                    hl, ha, hf = it, it - LAG, it - LAG - 1

                    if 0 <= hf < H:
                        # early O eviction: copy O rows + s row out of PSUM
                        # (frees the bank for this iteration's attention) and
                        # launch the 1/s bounce chain; brs lands well before
                        # the finish block at the end of this iteration
                        sf = st[hf]
                        ot_un = ot_pool.tile([DH, N], F16, name="ot_un", tag="ot_un")
                        nc.vector.tensor_copy(ot_un[:], sf["o"][0:DH, :])
                        rst = sm_pool.tile([1, N], F32, name="rst", tag="rst")
                        nc.vector.tensor_copy(rst[:], sf["o"][DH:VW, :])
                        s_d = dr_pool.tile([1, N], F32, name="s_d", tag="s_d")
                        nc.sync.dma_start(s_d[:], rst[:])
                        s2 = sm_pool.tile([TC, P], F32, name="s2", tag="s2")
                        nc.sync.dma_start(
                            s2[:], s_d[:].rearrange("o (q p) -> (o q) p", p=P)
                        )
                        s2h = sm_pool.tile([TC, P], F16, name="s2h", tag="s2h")
                        with nc.allow_low_precision(reason="1/s in fp16: 0.05% rel, well under tolerance"):
                            nc.vector.reciprocal(s2h[:], s2[:])
                        rs_d = dr_pool.tile([1, N], F16, name="rs_d", tag="rs_d")
                        nc.sync.dma_start(
                            rs_d[:].rearrange("o (q p) -> (o q) p", p=P), s2h[:]
                        )
                        brs_t = brs_pool.tile([DH, N], F16, name="brs", tag="brs")
                        nc.gpsimd.dma_start(brs_t[:], rs_d[:].to_broadcast((DH, N)))
                        sf["ot_un"], sf["brs"] = ot_un, brs_t

                    if hl < H:
                        st[hl] = {
                            "explid": [el_pool.tile([P, N], F16, name="explid", tag="explid") for _ in range(TC)],
                            "slc": sm_pool.tile([P, TC], F32, name="slc", tag="slc"),
                        }
                    if 0 <= ha < H:
                        sa = st[ha]
                        sa["o"] = ps_o.tile([VW, N], F32, name="o", tag="o")
                        e_ts = {}

                    for jc in range(TC + 1):
                        fc = QK_SCHED.get(it, {}).get(jc)
                        if fc is not None:
                            emit_qk_group(fc)
                        if it == 1 and jc < TC:
                            emit_v_group(jc)
                        yfc = YPAR_SCHED.get(it, {}).get(jc)
                        if yfc is not None:
                            emit_ypar_group(yfc)

                        if 0 <= ha < H and jc < TC:
                            # Lt = explid * bc (in place); mid = dots + Lt; E
                            expl = sa["explid"]
                            nc.vector.tensor_mul(expl[jc][:], expl[jc][:], sa["bc"][:])
                            mid = ps_sc.tile([P, N], F32, name="w", tag="w")
                            for ih in range(2):
                                nc.tensor.matmul(
                                    mid[:, ih * NH:(ih + 1) * NH],
                                    k_hs[ha][:, jc * P:(jc + 1) * P],
                                    q_hs[ha][:, ih * NH:(ih + 1) * NH],
                                    start=True, stop=False,
                                )
                                nc.tensor.matmul(
                                    mid[:, ih * NH:(ih + 1) * NH],
                                    id_sb[:],
                                    expl[jc][:, ih * NH:(ih + 1) * NH],
                                    start=False, stop=True,
                                )
                            e_t = e_pool.tile([P, N], F16, name="E", tag="E")
                            nc.scalar.activation(e_t[:], mid[:], EXP)
                            e_ts[jc] = e_t
                        if 0 <= ha < H and jc >= 1:
                            # vaug one step behind so PE never waits on exp
                            for ih in range(2):
                                nc.tensor.matmul(
                                    sa["o"][:, ih * NH:(ih + 1) * NH],
                                    v_sb[jc - 1][:, ha * VW:(ha + 1) * VW],
                                    e_ts[jc - 1][:, ih * NH:(ih + 1) * NH],
                                    start=(jc == 1), stop=(jc == TC),
                                )

                        if hl < H and jc < TC:
                            # lidar scores + biased exp w/ accumulate (sl rows
                            # via symmetry of lsim)
                            sl_ = st[hl]
                            pt = ps_sc.tile([P, N], F32, name="w", tag="w")
                            for ih in range(2):
                                nc.tensor.matmul(
                                    pt[:, ih * NH:(ih + 1) * NH],
                                    lid_hs[hl][:, jc * P:(jc + 1) * P],
                                    lid_hs[hl][:, ih * NH:(ih + 1) * NH],
                                    start=True, stop=True,
                                )
                            nc.scalar.activation(
                                sl_["explid"][jc][:], pt[:], EXP, bias=lb_sb[:],
                                accum_out=sl_["slc"][:, jc:jc + 1],
                            )

                    if 0 <= hf < H:
                        # finish: normalize the evicted O with broadcast 1/s
                        # (fp16 2x) and merge; brs was launched at iter top
                        sf = st[hf]
                        cf, offf = hf // 2, (hf % 2) * DH
                        ot_t = ot_pool.tile([DH, N], F16, name="ot", tag="ot")
                        nc.vector.tensor_mul(ot_t[:], sf["ot_un"][:], sf["brs"][:])
                        for ih in range(2):
                            mg = ps_sc.tile([P, NH], F32, name="w", tag="w")
                            nc.tensor.matmul(
                                mg[0:DH, 0:NH],
                                wm_sb[:],
                                ot_t[:, ih * NH:(ih + 1) * NH],
                                start=True, stop=True,
                            )
                            dst = om_sb[cf][offf:offf + DH, ih * NH:(ih + 1) * NH]
                            if need_bm:
                                nc.vector.tensor_scalar(
                                    out=dst, in0=mg[0:DH, 0:NH], scalar1=bm_sb[:],
                                    scalar2=None, op0=AX.add,
                                )
                            else:
                                nc.vector.tensor_copy(dst, mg[0:DH, 0:NH])
                        del st[hf]

                    if hl < H:
                        # bc chain: w1/sl column -> PE transpose -> DRAM
                        # bounce -> broadcast; consumed LAG iterations later
                        slc = st[hl]["slc"]
                        nc.vector.reciprocal(slc[:], slc[:])
                        nc.vector.tensor_scalar(
                            out=slc[:], in0=slc[:], scalar1=float(w1),
                            scalar2=None, op0=AX.mult,
                        )
                        rt = sm_pool.tile([TC, P], F16, name="rt", tag="rt")
                        tr = ps_sc.tile([P, NH], F32, name="w", tag="w")
                        nc.tensor.transpose(tr[0:TC, 0:P], slc[:], idf_sb[:])
                        nc.vector.tensor_copy(rt[:], tr[0:TC, 0:P])
                        rsl_d = dr_pool.tile([1, N], F16, name="rsl_d", tag="rsl_d")
                        nc.sync.dma_start(
                            rsl_d[:].rearrange("o (q p) -> (o q) p", p=P), rt[:]
                        )
                        bc_t = bc_pool.tile([P, N], F16, name="bc", tag="bc")
                        nc.sync.dma_start(
                            bc_t[0:64, :], rsl_d[:].to_broadcast((64, N))
                        )
                        nc.gpsimd.dma_start(
                            bc_t[64:P, :], rsl_d[:].to_broadcast((64, N))
                        )
                        st[hl]["bc"] = bc_t

            ctx_lp.__exit__(None, None, None)

            # ---------------- phase 3: final wout chunk + combine ----------------
            with tc.tile_pool(name="yout", bufs=2) as y_pool:
                for fc in range(KC):
                    pt = ps_sc.tile([P, N], F32, name="w", tag="w")
                    for ih in range(2):
                        nc.tensor.matmul(
                            pt[:, ih * NH:(ih + 1) * NH],
                            wo_sb[KC - 1][:, fc * P:(fc + 1) * P],
                            om_sb[KC - 1][:, ih * NH:(ih + 1) * NH],
                            start=True, stop=True,
                        )
                    yt = y_pool.tile([P, N], F32, name="yt", tag="yt")
                    nc.vector.tensor_add(yt[:], pt[:], ypar_sb[fc][:])
                    if need_bo:
                        nc.vector.tensor_scalar(
                            out=yt[:], in0=yt[:], scalar1=bo_sb[:, fc:fc + 1],
                            scalar2=None, op0=AX.add,
                        )
                    nc.sync.dma_start(y[fc * P:(fc + 1) * P, :], yt[:])

    nc.compile()
    return nc


def kernel(x, lidar, w_qkv, w_merge, b_merge, w_out, b_out, conv_w, conv_b, **_):
    x = np.asarray(x, np.float32)
    lidar = np.asarray(lidar, np.float32)
    w_qkv = np.asarray(w_qkv, np.float32)
    w_merge = np.asarray(w_merge, np.float32)
    b_merge = np.asarray(b_merge, np.float32)
    w_out = np.asarray(w_out, np.float32)
    b_out = np.asarray(b_out, np.float32)
    w0, w1 = float(np.asarray(conv_w)[0]), float(np.asarray(conv_w)[1])

    need_bm = bool(np.any(b_merge != 0))
    need_bo = bool(np.any(b_out != 0))
    key = (round(w1, 12), need_bm, need_bo)
    if key not in _cache:
        _cache.clear()
        _cache[key] = _build(w1, need_bm, need_bo)
    nc = _cache[key]

    # host-side weight prep: transposes + constant folds + fp16 casts
    wqkT = np.ascontiguousarray(w_qkv[0:QK].T)       # [512 dim, 1024 q|k feats]
    wqkT[:, 0:INNER] *= np.float32(SCALE * w0)       # fold w0*SCALE into q
    wqkT = wqkT.astype(np.float16)
    wvT = np.ascontiguousarray(w_qkv[QK:3 * INNER].T).astype(np.float16)
    wmT = np.ascontiguousarray(w_merge.T).astype(np.float16)
    woT = np.ascontiguousarray(w_out.T).astype(np.float16)
    identity = np.eye(P, dtype=np.float16)
    identityf = np.eye(P, dtype=np.float32)
    bm_c = np.ascontiguousarray(b_merge.reshape(DH, 1))
    bo_c = np.ascontiguousarray(b_out.reshape(KC, P).T)

    sqrt_scale = np.float32(SCALE ** 0.5)
    in_maps = []
    for b in range(B):
        in_maps.append({
            "xT": np.ascontiguousarray(x[b].T).astype(np.float16),
            "lidT": (lidar[b].T * sqrt_scale).astype(np.float16),
            "wqkT": wqkT,
            "wvT": wvT,
            "wmT": wmT,
            "woT": woT,
            "ident": identity,
            "identf": identityf,
            "onesv": np.ones((P, H, 1), np.float16),
            "bm": bm_c,
            "bo": bo_c,
        })

    try:
        res = run_bass_kernel_spmd(nc, in_maps, core_ids=list(range(B)))
    except Exception:
        # transient NRT device wedges recover on a fresh attempt
        import time as _time

        _time.sleep(5)
        res = run_bass_kernel_spmd(nc, in_maps, core_ids=list(range(B)))
    kernel._last_results = res

    out = np.stack([res.results[b]["y"].T for b in range(B)])
    return (out, lidar)
